# revision 1
# baseline (speedup 1.0000x reference)
"""Bidirectional LSTM over embedded event ids — Trainium2 Bass kernel.

Problem shapes (hardcoded): ids [32,64,256] int32, embed [6000,64],
per-direction LSTM E=H=64, output [32,64,256,128] f32.

Strategy: pure data parallel over the flattened B*S=2048 sequence axis,
256 sequences per core on 8 cores. On-device layout keeps the gate/hidden
dim on SBUF partitions and the sequence batch on the free dim, so the
recurrence z = Wcat.T @ [x_t; h_{t-1}] needs no transposes anywhere:

  rhs slot  [128, 256] f32r : parts 0:64 = x_t^T (DMA'd), 64:128 = h_{t-1}^T
  z PSUM    [128, 512]      : cols 0:256 = [i;f] rows, 256:512 = [g;o] rows
  sigmoid over the whole bank (g-weights pre-scaled by 2 so
  tanh(zg) = 2*sigmoid(2 zg) - 1 comes out of a fused affine-multiply)
  c update + h = o*tanh(c) as [64, 256] elementwise ops on parts 64:128.

h is written once, as float32r, directly into the next step's rhs slot;
the output DMA reads the same bytes. Host side does the embedding gather
(sequential-read layout for the device) and folds gate scaling into the
weights.
"""

import numpy as np

B, S, L, E, H, V = 32, 64, 256, 64, 64, 6000
NCORES = 8
NSEQ = B * S
NC_ = NSEQ // NCORES      # 256 sequences per core
GATES = 4 * H             # 256
KDIM = E + H              # 128

_CACHE = {}


def _build(l_steps, nc_seq, with_bias, prefetch=6, reps=1, gates_bf16=False,
           fc_on="pool", tail_prio=0, sigma_split=False,
           out_dma="sync"):
    import concourse.bacc as bacc
    import concourse.tile as tile
    from concourse import mybir

    dt = mybir.dt
    AF = mybir.ActivationFunctionType
    DIRS = ("f", "b")

    nc = bacc.Bacc("TRN2", num_devices=NCORES, debug=False)
    x_d = nc.dram_tensor("x", (E, l_steps, nc_seq), dt.float32r,
                         kind="ExternalInput")
    xr_d = nc.dram_tensor("xr", (E, l_steps, nc_seq), dt.float32r,
                          kind="ExternalInput")
    z0_d = nc.dram_tensor("z0", (H, nc_seq), dt.float32r,
                          kind="ExternalInput")
    w_d = {d: nc.dram_tensor(f"w_{d}", (KDIM, GATES), dt.float32r,
                             kind="ExternalInput") for d in DIRS}
    bias_d = {}
    if with_bias:
        for d in DIRS:
            bias_d[d] = nc.dram_tensor(f"bias_{d}", (KDIM, 2), dt.float32,
                                       kind="ExternalInput")
    o_d = {d: nc.dram_tensor(f"o_{d}", (H, l_steps, nc_seq), dt.float32r,
                             kind="ExternalOutput") for d in DIRS}


    with tile.TileContext(nc) as tc:
        with (
            tc.tile_pool(name="singles", bufs=1) as singles,
            tc.tile_pool(name="rhs", bufs=prefetch + 3) as rhs_pool,
            tc.tile_pool(name="zs", bufs=3) as zs_pool,
            tc.tile_pool(name="tmp", bufs=3) as tmp_pool,
            tc.tile_pool(name="psum_f", bufs=2, space="PSUM") as psum_f,
            tc.tile_pool(name="psum_b", bufs=2, space="PSUM") as psum_b,
        ):
            psum_pool = {"f": psum_f, "b": psum_b}
            w_t = {}
            bias_t = {}
            c_t = {}
            tc_t = {}
            for d in DIRS:
                c_t[d] = singles.tile([128, nc_seq], dt.float32,
                                      name=f"c_{d}", tag=f"c_{d}")
                nc.vector.memset(c_t[d][64:128, :], 0.0)
                tc_t[d] = singles.tile([128, nc_seq], dt.float32,
                                       name=f"tcv_{d}", tag=f"tcv_{d}")
            for d in DIRS:
                w_t[d] = singles.tile([KDIM, GATES], dt.float32r,
                                      name=f"w_{d}", tag=f"w_{d}")
                nc.sync.dma_start(out=w_t[d][:, :], in_=w_d[d].ap())
                if with_bias:
                    bias_t[d] = singles.tile([KDIM, 2], dt.float32,
                                             name=f"biast_{d}", tag=f"bias_{d}")
                    nc.sync.dma_start(out=bias_t[d][:, :], in_=bias_d[d].ap())
            rhs_tiles = {d: {} for d in DIRS}

            def new_slot(d, t):
                tl = rhs_pool.tile([128, nc_seq], dt.float32r,
                                   name=f"rhs_{d}", tag=f"rhs_{d}")
                rhs_tiles[d][t] = tl
                if t < l_steps:
                    src_t = x_d if d == "f" else xr_d
                    nc.sync.dma_start(out=tl[0:64, :],
                                      in_=src_t.ap()[:, t, :])
                return tl

            for d in DIRS:
                for tt in range(min(prefetch, l_steps + 1)):
                    new_slot(d, tt)
                nc.sync.dma_start(out=rhs_tiles[d][0][64:128, :],
                                  in_=z0_d.ap())

            # both dirs: blockA=[i;f], blockB=[g';o]; cell state rows
            # 64:128; the only cross-quadrant access is ig's upward write
            # (reads @0:64, writes @64:128), which is HW-verified
            A, B = slice(0, 64), slice(64, 128)
            ROWS = {"f": {"c": B, "f": B, "i": A, "o": B, "g": A},
                    "b": {"c": B, "f": B, "i": A, "o": B, "g": A}}
            zdt = dt.bfloat16 if gates_bf16 else dt.float32

            for rep in range(reps):
              for t in range(l_steps):
                zs_t = {}
                for d in DIRS:
                    r = ROWS[d]
                    if t + prefetch <= l_steps:
                        new_slot(d, t + prefetch)
                    rhs = rhs_tiles[d][t][:, :]
                    z = psum_pool[d].tile([128, 512], dt.float32,
                                          name=f"z_{d}", tag=f"z_{d}")
                    nc.tensor.matmul(z[:, 0:nc_seq], w_t[d][:, 0:128],
                                     rhs, start=True, stop=True)
                    nc.tensor.matmul(z[:, nc_seq:2 * nc_seq],
                                     w_t[d][:, 128:256],
                                     rhs, start=True, stop=True)
                    zs = zs_pool.tile([128, 512], zdt,
                                       name=f"zs_{d}", tag=f"zs_{d}")
                    zs_t[d] = zs
                    if with_bias:
                        nc.scalar.activation(zs[:, 0:nc_seq], z[:, 0:nc_seq],
                                             AF.Sigmoid,
                                             bias=bias_t[d][:, 0:1])
                        nc.scalar.activation(zs[:, nc_seq:2 * nc_seq],
                                             z[:, nc_seq:2 * nc_seq],
                                             AF.Sigmoid,
                                             bias=bias_t[d][:, 1:2])
                    else:
                        nc.scalar.activation(zs[:, :], z[:, :], AF.Sigmoid)
                    # g = tanh(zg) = 2*sig(2 zg) - 1
                    gg = tmp_pool.tile([128, nc_seq], zdt,
                                       name=f"gg_{d}", tag=f"gg_{d}")
                    nc.vector.tensor_scalar(
                        out=gg[r["g"], :],
                        in0=zs[r["g"], nc_seq:2 * nc_seq],
                        scalar1=2.0, scalar2=1.0,
                        op0=mybir.AluOpType.mult,
                        op1=mybir.AluOpType.subtract)
                    t1 = tmp_pool.tile([128, nc_seq], zdt,
                                       name=f"t1_{d}", tag=f"t1_{d}")
                    nc.vector.tensor_mul(t1[r["c"], :], gg[r["g"], :],
                                         zs[r["i"], 0:nc_seq])
                    # t2 = sig(zf) * c
                    t2 = tmp_pool.tile([128, nc_seq], dt.float32,
                                       name=f"t2_{d}", tag=f"t2_{d}")
                    fc_eng = nc.gpsimd if fc_on == "pool" else nc.vector
                    fc_eng.tensor_mul(t2[r["c"], :],
                                      zs[r["f"], 0:nc_seq],
                                      c_t[d][r["c"], :])
                    nc.vector.tensor_add(c_t[d][r["c"], :],
                                         t1[r["c"], :], t2[r["c"], :])
                    # per-dir tanh keeps the two chains decoupled
                    nc.scalar.activation(tc_t[d][r["c"], :],
                                         c_t[d][r["c"], :], AF.Tanh)
                    nxt = rhs_tiles[d][t + 1]
                    nc.vector.tensor_mul(nxt[64:128, :],
                                         zs[r["o"], nc_seq:2 * nc_seq],
                                         tc_t[d][r["c"], :])
                    out_eng = nc.scalar if out_dma == "act" else nc.sync
                    out_eng.dma_start(out=o_d[d].ap()[:, t, :],
                                      in_=nxt[64:128, :])
                    del rhs_tiles[d][t]

    nc.compile()
    return nc


def _get_nc(l_steps, nc_seq, with_bias):
    key = (l_steps, nc_seq, with_bias)
    if key not in _CACHE:
        _CACHE[key] = _build(l_steps, nc_seq, with_bias)
    return _CACHE[key]


def _prep_w(Wk, Wr, b, mirror=False):
    """[128, 256] f32 contiguous: rows = [x-proj; h-proj], g-gate cols
    pre-scaled by 2 (tanh-via-sigmoid). Keras col order is i,f,g,o;
    device blockA/blockB layouts are [i,f | 2g,o], or mirrored
    [f,i | o,2g] for the fwd direction (see ROWS in _build).
    Returns (Wcat, bias[128,2])."""
    Wcat = np.concatenate([np.asarray(Wk), np.asarray(Wr)], axis=0)
    b = np.asarray(b)
    i_, f_, g_, o_ = (Wcat[:, 0:64], Wcat[:, 64:128],
                      2.0 * Wcat[:, 128:192], Wcat[:, 192:256])
    bi, bf, bg, bo = b[0:64], b[64:128], 2.0 * b[128:192], b[192:256]
    if mirror:
        cols = [f_, i_, o_, g_]
        bcols = [np.concatenate([bf, bi]), np.concatenate([bo, bg])]
    else:
        cols = [i_, f_, g_, o_]
        bcols = [np.concatenate([bi, bf]), np.concatenate([bg, bo])]
    Wout = np.ascontiguousarray(np.concatenate(cols, axis=1),
                                dtype=np.float32)
    bias = None
    if np.any(b != 0.0):
        bias = np.ascontiguousarray(np.stack(bcols, axis=1),
                                    dtype=np.float32)
    return Wout, bias


def kernel(ids, embed_table, Wk_f, Wr_f, b_f, Wk_b, Wr_b, b_b):
    from concourse import bass_utils

    ids = np.asarray(ids)
    embed_table = np.asarray(embed_table, dtype=np.float32)
    wf, bias_f = _prep_w(Wk_f, Wr_f, b_f, mirror=False)
    wb, bias_b = _prep_w(Wk_b, Wr_b, b_b, mirror=False)
    with_bias = bias_f is not None or bias_b is not None
    if with_bias:
        if bias_f is None:
            bias_f = np.zeros((KDIM, 2), np.float32)
        if bias_b is None:
            bias_b = np.zeros((KDIM, 2), np.float32)

    nc = _get_nc(L, NC_, with_bias)

    ids2 = ids.reshape(NSEQ, L)
    in_maps = []
    for m in range(NCORES):
        idc = ids2[m * NC_:(m + 1) * NC_]            # [NC_, L]
        xc = embed_table[idc]                        # [NC_, L, E]
        xT = np.ascontiguousarray(xc.transpose(2, 1, 0))  # [E, L, NC_]
        im = {"x": xT, "xr": np.ascontiguousarray(xT[:, ::-1]),
              "w_f": wf, "w_b": wb,
              "z0": np.zeros((H, NC_), np.float32)}
        if with_bias:
            im["bias_f"] = bias_f
            im["bias_b"] = bias_b
        in_maps.append(im)

    res = bass_utils.run_bass_kernel_spmd(nc, in_maps,
                                          core_ids=list(range(NCORES)))

    out = np.empty((NSEQ, L, 2 * H), dtype=np.float32)
    for m in range(NCORES):
        hf = res.results[m]["o_f"]                   # [H, L, NC_]
        hb = res.results[m]["o_b"][:, ::-1, :]       # iteration -> time order
        sl = slice(m * NC_, (m + 1) * NC_)
        out[sl, :, 0:H] = hf.transpose(2, 1, 0)
        out[sl, :, H:2 * H] = hb.transpose(2, 1, 0)
    return out.reshape(B, S, L, 2 * H)



# revision 4
# speedup vs baseline: 1.6211x; 1.6211x over previous
"""Bidirectional LSTM over embedded event ids — Trainium2 Bass kernel.

Problem shapes (hardcoded): ids [32,64,256] int32, embed [6000,64],
per-direction LSTM E=H=64, output [32,64,256,128] f32.

Strategy: direction-parallel + data-parallel. Cores 0-3 run the forward
LSTM on sequence quarters 0-3 (512 seqs each); cores 4-7 run the backward
LSTM on the same quarters (host pre-reverses time). Per core, the 512
sequences are packed two-per-partition-lane: partition p = hdim + 64*(s
>= 256), free column j = s % 256. All per-step tensors are [128, 256]
with full partition utilization.

The 256 free columns split into G=3 interleaved groups (86/85/85) whose
recurrence chains overlap on the engines — the serial chain (matmul ->
sigmoid -> cell update -> tanh -> h-mul) of one group hides behind the
activation work of the other two.

Per group per step:
  - 8 matmuls (4 gates x {x-part, h-part}), bf16, accumulate in PSUM
    Z[128, 4f]; weights are [128,128] blockdiag(Wq, Wq) so both
    sequence halves share one matmul. g-gate weights pre-scaled by 2.
  - one Sigmoid over all 4 gates: S = sigma(Z)   (tanh(zg) = 2*sig(2 zg)-1)
  - t1 = (S_g - 0.5) * S_i           (scalar_tensor_tensor, Pool)
  - t2 = S_f * c                     (tensor_tensor, DVE)
  - c  = 2*t1 + t2                   (scalar_tensor_tensor, DVE)
  - Tc = tanh(c)                     (Act; same act table as sigmoid)
  - h  = Tc * S_o -> bf16 h-ring     (tensor_tensor, DVE)

x is streamed from HBM in T-step blocks into a 3T-slot bf16 ring; h is
written to a 2T-slot bf16 ring that doubles as matmul rhs source and
output staging (one batched DMA per T steps each way).
"""

import numpy as np
import ml_dtypes

B, S, L, E, H, V = 32, 64, 256, 64, 64, 6000
NCORES = 8
NSEQ = B * S               # 2048
NC_SEQ = 512               # sequences per core (one direction)
NQ = NSEQ // NC_SEQ        # 4 sequence quarters
COLS = NC_SEQ // 2         # 256 free columns (2 seqs per partition lane)
GB = [0, 86, 171, 256]     # group boundaries over free columns
NG = 3
T = 16                     # DMA block (timesteps)
RX = 3 * T                 # x ring slots
RH = 2 * T                 # h ring slots
NB = L // T

_CACHE = {}
_BF16 = ml_dtypes.bfloat16


def _build(with_bias):
    import concourse.bacc as bacc
    import concourse.tile as tile
    from concourse import mybir

    dt = mybir.dt
    AF = mybir.ActivationFunctionType
    OP = mybir.AluOpType

    nc = bacc.Bacc("TRN2", num_devices=NCORES, debug=False)
    xc_d = nc.dram_tensor("xc", (128, L * COLS), dt.bfloat16,
                          kind="ExternalInput")
    wx_d = nc.dram_tensor("wx", (128, 512), dt.bfloat16, kind="ExternalInput")
    wh_d = nc.dram_tensor("wh", (128, 512), dt.bfloat16, kind="ExternalInput")
    if with_bias:
        wb_d = nc.dram_tensor("wb", (128, 512), dt.bfloat16,
                              kind="ExternalInput")
    out_d = nc.dram_tensor("out", (128, L * COLS), dt.bfloat16,
                           kind="ExternalOutput")

    with tile.TileContext(nc) as tc:
        with (
            tc.tile_pool(name="big", bufs=1) as big,
            tc.tile_pool(name="zp", bufs=2, space="PSUM") as zp,
        ):
            XR = big.tile([128, RX * COLS], dt.bfloat16, name="XR", tag="XR")
            HR = big.tile([128, RH * COLS], dt.bfloat16, name="HR", tag="HR")
            wxt = big.tile([128, 512], dt.bfloat16, name="wxt", tag="wxt")
            wht = big.tile([128, 512], dt.bfloat16, name="wht", tag="wht")
            nc.sync.dma_start(out=wxt[:, :], in_=wx_d.ap())
            nc.sync.dma_start(out=wht[:, :], in_=wh_d.ap())
            if with_bias:
                wbt = big.tile([128, 512], dt.bfloat16, name="wbt", tag="wbt")
                nc.sync.dma_start(out=wbt[:, :], in_=wb_d.ap())
                ones = big.tile([128, COLS], dt.bfloat16, name="ones",
                                tag="ones")
                nc.vector.memset(ones[:, :], 1.0)
            FS = [GB[g + 1] - GB[g] for g in range(NG)]
            S_t, c_t, Tc_t, t1_t, t2_t = {}, {}, {}, {}, {}
            for g in range(NG):
                f = FS[g]
                S_t[g] = big.tile([128, 4 * f], dt.float32, name=f"S{g}",
                                  tag=f"S{g}")
                c_t[g] = big.tile([128, f], dt.float32, name=f"c{g}",
                                  tag=f"c{g}")
                nc.vector.memset(c_t[g][:, :], 0.0)
                Tc_t[g] = big.tile([128, f], dt.float32, name=f"Tc{g}",
                                   tag=f"Tc{g}")
                t1_t[g] = big.tile([128, f], dt.float32, name=f"t1{g}",
                                   tag=f"t1{g}")
                t2_t[g] = big.tile([128, f], dt.float32, name=f"t2{g}",
                                   tag=f"t2{g}")

            def xdma(b):
                lo = (b % 3) * T * COLS
                nc.sync.dma_start(out=XR[:, lo:lo + T * COLS],
                                  in_=xc_d.ap()[:, b * T * COLS:
                                                (b + 1) * T * COLS])

            for b in range(min(3, NB)):
                xdma(b)

            for t in range(L):
                zt = {}
                for g in range(NG):
                    zt[g] = zp.tile([128, 4 * FS[g]], dt.float32,
                                    name=f"z{g}", tag=f"z{g}")
                xs = (t % RX) * COLS
                hp = ((t - 1) % RH) * COLS
                hs = (t % RH) * COLS
                # x-part (+ bias) matmuls: independent of the recurrence.
                # start=True only on the first matmul into each psum tile:
                # start marks the whole 2KB zero-region pending-zero, so
                # later writes overwrite-as-zero once and then accumulate.
                for g in range(NG):
                    f, a = FS[g], GB[g]
                    rhs = XR[:, xs + a:xs + a + f]
                    for q in range(4):
                        last = (t == 0) and not with_bias and q == 3
                        nc.tensor.matmul(zt[g][:, q * f:(q + 1) * f],
                                         wxt[:, q * 128:(q + 1) * 128],
                                         rhs, start=(q == 0), stop=last)
                    if with_bias:
                        for q in range(4):
                            nc.tensor.matmul(zt[g][:, q * f:(q + 1) * f],
                                             wbt[:, q * 128:(q + 1) * 128],
                                             ones[:, a:a + f], start=False,
                                             stop=(t == 0) and q == 3)
                # h-part matmuls: the recurrence-critical ones
                if t > 0:
                    for g in range(NG):
                        f, a = FS[g], GB[g]
                        rhs = HR[:, hp + a:hp + a + f]
                        for q in range(4):
                            nc.tensor.matmul(zt[g][:, q * f:(q + 1) * f],
                                             wht[:, q * 128:(q + 1) * 128],
                                             rhs, start=False, stop=(q == 3))
                for g in range(NG):
                    nc.scalar.activation(S_t[g][:, :], zt[g][:, :],
                                         AF.Sigmoid)
                for g in range(NG):
                    f = FS[g]
                    nc.gpsimd.tensor_tensor(t2_t[g][:, :],
                                            S_t[g][:, f:2 * f],
                                            c_t[g][:, :], OP.mult)
                for g in range(NG):
                    f = FS[g]
                    nc.vector.scalar_tensor_tensor(
                        out=t1_t[g][:, :], in0=S_t[g][:, 2 * f:3 * f],
                        scalar=0.5, in1=S_t[g][:, 0:f],
                        op0=OP.subtract, op1=OP.mult)
                    nc.vector.scalar_tensor_tensor(
                        out=c_t[g][:, :], in0=t1_t[g][:, :], scalar=2.0,
                        in1=t2_t[g][:, :], op0=OP.mult, op1=OP.add)
                for g in range(NG):
                    nc.scalar.activation(Tc_t[g][:, :], c_t[g][:, :],
                                         AF.Tanh)
                for g in range(NG):
                    f, a = FS[g], GB[g]
                    nc.vector.tensor_tensor(HR[:, hs + a:hs + a + f],
                                            Tc_t[g][:, :],
                                            S_t[g][:, 3 * f:4 * f], OP.mult)
                if t % T == 1 and t // T >= 1 and t // T + 2 < NB:
                    xdma(t // T + 2)
                if t % T == T - 1:
                    k = t // T
                    lo = (k % 2) * T * COLS
                    nc.sync.dma_start(
                        out=out_d.ap()[:, k * T * COLS:(k + 1) * T * COLS],
                        in_=HR[:, lo:lo + T * COLS])

    nc.compile()
    return nc


def _get_nc(with_bias):
    key = ("v6", with_bias)
    if key not in _CACHE:
        _CACHE[key] = _build(with_bias)
    return _CACHE[key]


def _prep_w(Wk, Wr, b):
    """Blockdiag-packed lhsT weights [128, 4*128] bf16 for x- and h-parts,
    plus optional rank-1 bias lhsT. Gate order i,f,g,o; g pre-scaled by 2
    (tanh via sigmoid)."""
    Wk = np.asarray(Wk, np.float32)
    Wr = np.asarray(Wr, np.float32)
    b = np.asarray(b, np.float32)
    wx = np.zeros((128, 512), np.float32)
    wh = np.zeros((128, 512), np.float32)
    wb = np.zeros((128, 512), np.float32)
    for q in range(4):
        sc = 2.0 if q == 2 else 1.0
        Wq = Wk[:, q * 64:(q + 1) * 64] * sc
        Rq = Wr[:, q * 64:(q + 1) * 64] * sc
        wx[0:64, q * 128 + 0:q * 128 + 64] = Wq
        wx[64:128, q * 128 + 64:q * 128 + 128] = Wq
        wh[0:64, q * 128 + 0:q * 128 + 64] = Rq
        wh[64:128, q * 128 + 64:q * 128 + 128] = Rq
        bq = b[q * 64:(q + 1) * 64] * sc
        wb[0, q * 128 + 0:q * 128 + 64] = bq
        wb[0, q * 128 + 64:q * 128 + 128] = bq
    with_bias = bool(np.any(b != 0.0))
    return (wx.astype(_BF16), wh.astype(_BF16), wb.astype(_BF16), with_bias)


def _pack_x(ids_q, emb, rev):
    """ids_q [512, L] -> packed [128, L*COLS] bf16 (p = hdim + 64*(s>=256))."""
    x = emb[ids_q]                                   # [512, L, E] f32
    if rev:
        x = x[:, ::-1, :]
    xr = x.reshape(2, COLS, L, E).transpose(0, 3, 2, 1)   # [2, E, L, COLS]
    return np.ascontiguousarray(xr).reshape(128, L * COLS).astype(_BF16)


def _unpack_h(o, rev):
    """[128, L*COLS] -> h [512, L, 64] f32."""
    o = np.asarray(o, np.float32).reshape(2, 64, L, COLS)
    h = o.transpose(0, 3, 2, 1).reshape(NC_SEQ, L, 64)
    if rev:
        h = h[:, ::-1, :]
    return h


def _in_maps(ids, embed_table, Wk_f, Wr_f, b_f, Wk_b, Wr_b, b_b):
    ids2 = np.asarray(ids).reshape(NSEQ, L)
    emb = np.asarray(embed_table, dtype=np.float32)
    wx_f, wh_f, wb_f, bias_f = _prep_w(Wk_f, Wr_f, b_f)
    wx_b, wh_b, wb_b, bias_b = _prep_w(Wk_b, Wr_b, b_b)
    with_bias = bias_f or bias_b
    in_maps = []
    for m in range(NCORES):
        rev = m >= NQ
        q = m % NQ
        ids_q = ids2[q * NC_SEQ:(q + 1) * NC_SEQ]
        im = {"xc": _pack_x(ids_q, emb, rev),
              "wx": wx_b if rev else wx_f,
              "wh": wh_b if rev else wh_f}
        if with_bias:
            im["wb"] = wb_b if rev else wb_f
        in_maps.append(im)
    return in_maps, with_bias


def kernel(ids, embed_table, Wk_f, Wr_f, b_f, Wk_b, Wr_b, b_b):
    from concourse import bass_utils

    in_maps, with_bias = _in_maps(ids, embed_table, Wk_f, Wr_f, b_f,
                                  Wk_b, Wr_b, b_b)
    nc = _get_nc(with_bias)
    res = bass_utils.run_bass_kernel_spmd(nc, in_maps,
                                          core_ids=list(range(NCORES)))

    out = np.empty((NSEQ, L, 2 * H), dtype=np.float32)
    for m in range(NCORES):
        rev = m >= NQ
        q = m % NQ
        h = _unpack_h(res.results[m]["out"], rev)
        sl = slice(q * NC_SEQ, (q + 1) * NC_SEQ)
        if rev:
            out[sl, :, H:2 * H] = h
        else:
            out[sl, :, 0:H] = h
    return out.reshape(B, S, L, 2 * H)
